# revision 25
# baseline (speedup 1.0000x reference)
"""Trainium2 Bass kernel for nn_PolynomialLearningRule.

Math:  delta_w = sum_i pw[i]/B * ((y^ey s^es).T @ x^ex) * w^ew   over 35 tuples
       delta_b = sum_i pb[i]/B * colsum(y^ey) * b^eb             over 10 tuples

Decomposition (validated in numpy): group by k=ew into delta_w = sum_k w^k * G_k.
Triples (ex,es,ey) with es=ey=0 reduce to row-broadcast colsums of x^ex; triples
with ex=0 reduce to column-broadcast colsums of y^ey s^es; only 7 triples need
dense [512,512] matmuls.

Precision/speed: fp32 matmuls run 4 cycles/row on the TRN2 PE (LOW_HIGH double
pass at half rate).  Instead every matmul operand A is represented as an fp16
pair (Ah = fp16(A), Am = fp16(A - Ah)); A.T@X is computed as Ah@Xh + Ah@Xm +
Am@Xh in one PSUM accumulation group (3 passes at full PE rate, ~1.3x the
fp32-instruction cost at 3x fewer cycles).  HW-measured error of this scheme is
~2e-7 — fp32-grade.  Raw x/s/y splits come pre-computed from the host; derived
powers are squared/multiplied in fp32 on ACT/DVE and split on the fly.

Distribution (8 cores): batch-sharded compute; a slab-interleaved ReduceScatter
hands rank r its 64 output rows of D0/D1 plus (pre-sliced) u vectors and full
g/v vectors; each core Horner-combines only its rows; host concatenates.
"""

import numpy as np
from itertools import product

B_TOTAL = 8192
D = 512
P = 128
N_CORES = 8


def _gen_exps(num_vars, degree):
    return [e for e in product(range(degree + 1), repeat=num_vars) if sum(e) <= degree]


EXPS_W = _gen_exps(4, 3)  # (ex, es, ey, ew)
EXPS_B = _gen_exps(2, 3)  # (ey, eb)
IDX_W = {t: i for i, t in enumerate(EXPS_W)}
IDX_B = {t: i for i, t in enumerate(EXPS_B)}

# dense triples (ex>=1 and (es,ey)!=(0,0)); lhsT source and rhs power per triple
DENSE = [(1, 0, 1), (1, 1, 0), (1, 0, 2), (1, 1, 1), (1, 2, 0), (2, 0, 1), (2, 1, 0)]
DENSE_LHS = ["y", "s", "y2", "ys", "s2", "y", "s"]
DENSE_RHS = ["x", "x", "x", "x", "x", "x2", "x2"]
# colsum streams, row order = (es,ey) pairs then x powers
COLP = [(0, 1), (0, 2), (0, 3), (1, 0), (1, 1), (1, 2), (2, 0), (2, 1), (3, 0)]
COL_SRC = ["y", "y2", "y3", "s", "ys", "y2s", "s2", "ys2", "s3", "x", "x2", "x3"]


def host_tables(params_w, params_b, n_cores):
    """coefs [1,16] and ugv [13,16] device inputs from runtime params."""
    pw = params_w.astype(np.float64) / B_TOTAL
    pb = params_b.astype(np.float64) / B_TOTAL

    def c(ex, es, ey, ew):
        t = (ex, es, ey, ew)
        return pw[IDX_W[t]] if t in IDX_W else 0.0

    coefs = np.zeros((1, 16), np.float32)
    for ti, t in enumerate(DENSE):
        coefs[0, ti] = c(*t, 0)
    coefs[0, 7] = c(*DENSE[0], 1)
    coefs[0, 8] = c(*DENSE[1], 1)
    coefs[0, 9] = params_w[IDX_W[(0, 0, 0, 3)]]  # alpha3

    ugv = np.zeros((13, 16), np.float32)
    for p_i, (es, ey) in enumerate(COLP):
        for k in range(4):
            ugv[p_i, k] = c(0, es, ey, k)          # u_k
        if es == 0:
            for k in range(4):
                t = (ey, k)
                if t in IDX_B:
                    ugv[p_i, 4 + k] = pb[IDX_B[t]]  # g_k
    for e in range(1, 4):
        for k in range(3):
            ugv[9 + e - 1, 8 + k] = c(e, 0, 0, k)   # v_k
    for k in range(3):  # constants via all-ones cp row 12 (sum multiplies by n_cores)
        ugv[12, k] = params_w[IDX_W[(0, 0, 0, k)]] / n_cores
    for k in range(4):
        t = (0, k)
        if t in IDX_B:
            ugv[12, 4 + k] = params_b[IDX_B[t]] / n_cores
    return coefs, ugv


def build_nc(n_cores, Bs):
    import concourse.bass as bass
    import concourse.mybir as mybir
    import concourse.tile as tile
    from concourse import bacc
    from concourse.masks import make_identity

    f32 = mybir.dt.float32
    f16 = mybir.dt.float16
    ADD = mybir.AluOpType.add
    SUB = mybir.AluOpType.subtract
    MULT = mybir.AluOpType.mult
    SQUARE = mybir.ActivationFunctionType.Square
    COPY = mybir.ActivationFunctionType.Copy
    nbt = Bs // P

    RPC = D // n_cores           # dw rows combined per core
    CH = min(RPC, P)             # partition chunk for the combine phase
    nch = RPC // CH
    SLAB = 2 * RPC + 10          # D0 | D1 | u(3, RPC cols) | g(4) | v(3)

    nc = bacc.Bacc(
        "TRN2", target_bir_lowering=False, debug=False, num_devices=n_cores
    )

    # raw inputs pre-split into fp16 hi/mid pairs on the host
    split_d = {}
    for nm in ["xh", "xm", "yh", "ym", "sh", "sm"]:
        split_d[nm] = nc.dram_tensor(nm, [Bs, D], f16, kind="ExternalInput")
    xf_d = nc.dram_tensor("xf", [Bs, D], f32, kind="ExternalInput")
    yf_d = nc.dram_tensor("yf", [Bs, D], f32, kind="ExternalInput")
    sf_d = nc.dram_tensor("sf", [Bs, D], f32, kind="ExternalInput")
    w_d = nc.dram_tensor("w", [RPC, D], f32, kind="ExternalInput")  # per-core slab
    b_d = nc.dram_tensor("b", [1, D], f32, kind="ExternalInput")
    coefs_d = nc.dram_tensor("coefs", [1, 16], f32, kind="ExternalInput")
    ugv_d = nc.dram_tensor("ugv", [13, 16], f32, kind="ExternalInput")
    eye12_d = nc.dram_tensor("eye12", [12, 12], f16, kind="ExternalInput")
    dw_d = nc.dram_tensor("dw", [RPC, D], f32, kind="ExternalOutput")
    db_d = nc.dram_tensor("db", [1, D], f32, kind="ExternalOutput")
    part_d = nc.dram_tensor("partials", [n_cores * SLAB, D], f32)
    part_r = nc.dram_tensor("partials_red", [SLAB, D], f32)
    pview = part_d.ap().rearrange("(r s) d -> r s d", s=SLAB)

    with tile.TileContext(nc) as tc:
        with (
            tc.tile_pool(name="persist", bufs=1) as persist,
            tc.tile_pool(name="scratch", bufs=2) as scratch,
            tc.tile_pool(name="ps_dense", bufs=4, space="PSUM") as ps_dense,
            tc.tile_pool(name="ps_cs", bufs=1, space="PSUM") as ps_cs_pool,
            tc.tile_pool(name="ps_small", bufs=2, space="PSUM") as ps_small,
        ):
            # ---- constants / small inputs ----
            # per-stream selector: column r of sel[r] is all-ones -> matmul
            # accumulates colsum(rhs) into psum row r, +=0 elsewhere
            sels = []
            for p_i in range(12):
                sp = persist.tile([P, 12], f16, name=f"sel_{p_i}", tag=f"sel_{p_i}")
                nc.sync.dma_start(
                    sp, eye12_d[p_i : p_i + 1, :].to_broadcast((P, 12))
                )
                sels.append(sp)

            coef_bc = persist.tile([P, 16], f32, name="coef_bc")
            nc.sync.dma_start(coef_bc, coefs_d[0:1, :].to_broadcast((P, 16)))

            ugv_sb = persist.tile([P, 16], f32, name="ugv_sb")
            nc.gpsimd.memset(ugv_sb, 0.0)
            nc.sync.dma_start(ugv_sb[0:13, :], ugv_d[:, :])

            b_sb = persist.tile([1, D], f32, name="b_sb")
            nc.sync.dma_start(b_sb, b_d[:, :])

            wt = []
            for j in range(nch):
                wj = persist.tile([CH, D], f32, name=f"w_{j}", tag=f"w_{j}")
                nc.sync.dma_start(wj, w_d[j * CH : (j + 1) * CH, :])
                wt.append(wj)

            identity = persist.tile([P, P], f32, name="identity")
            make_identity(nc, identity)

            # rows 12-127 all-ones; only row 12 of ugv is nonzero there, so the
            # extra ones rows contribute nothing to the vector-fold matmul
            cp_sb = persist.tile([P, D], f32, name="cp_sb")
            nc.gpsimd.memset(cp_sb, 1.0)
            uin = persist.tile([P, RPC], f32, name="uin")
            nc.gpsimd.memset(uin, 0.0)

            # PE pre-touch: consume the gpsimd-built constants once so later
            # matmuls don't need a second (Pool) sync wait
            ps_warm = ps_small.tile([P, P], f32, name="ps_warm", tag="small")
            nc.tensor.matmul(ps_warm, identity, cp_sb[:, 0:P], start=True, stop=True)

            # ---- per-bt: load, derive powers, split, colsum-stream ----
            # persistent fp16 split pairs needed by the dense phase
            SETS = ["x", "x2", "y", "s", "y2", "s2", "ys"]
            hi = {n: [] for n in SETS}
            mi = {n: [] for n in SETS}
            ps_cs = ps_cs_pool.tile([12, D], f32, name="ps_cs")
            n_cs = 24 * nbt
            i_cs = 0

            def cs_mm(row, tile_):
                nonlocal i_cs
                nc.tensor.matmul(
                    ps_cs, sels[row], tile_,
                    start=(i_cs == 0), stop=(i_cs == n_cs - 1),
                )
                i_cs += 1

            for bt in range(nbt):
                # raw split pairs straight from DRAM
                for src, pfx in [("x", "x"), ("y", "y"), ("s", "s")]:
                    th = persist.tile([P, D], f16, name=f"{pfx}h_{bt}", tag=f"{pfx}h_{bt}")
                    nc.sync.dma_start(th, split_d[pfx + "h"][bt * P : (bt + 1) * P, :])
                    hi[src].append(th)
                    tm = persist.tile([P, D], f16, name=f"{pfx}m_{bt}", tag=f"{pfx}m_{bt}")
                    nc.sync.dma_start(tm, split_d[pfx + "m"][bt * P : (bt + 1) * P, :])
                    mi[src].append(tm)
                xf = scratch.tile([P, D], f32, name=f"xf_{bt}", tag="xf")
                nc.sync.dma_start(xf, xf_d[bt * P : (bt + 1) * P, :])
                yf = scratch.tile([P, D], f32, name=f"yf_{bt}", tag="yf")
                nc.sync.dma_start(yf, yf_d[bt * P : (bt + 1) * P, :])
                sf = scratch.tile([P, D], f32, name=f"sf_{bt}", tag="sf")
                nc.sync.dma_start(sf, sf_d[bt * P : (bt + 1) * P, :])

                # derived powers in fp32 (ACT squares / DVE mult)
                x2f = scratch.tile([P, D], f32, name=f"x2f_{bt}", tag="x2f")
                nc.scalar.activation(x2f, xf, SQUARE)
                y2f = scratch.tile([P, D], f32, name=f"y2f_{bt}", tag="y2f")
                nc.scalar.activation(y2f, yf, SQUARE)
                s2f = scratch.tile([P, D], f32, name=f"s2f_{bt}", tag="s2f")
                nc.scalar.activation(s2f, sf, SQUARE)
                ysf = scratch.tile([P, D], f32, name=f"ysf_{bt}", tag="ysf")
                nc.vector.tensor_tensor(ysf, yf, sf, MULT)

                # split the stored derived sets (hi on ACT, mid on DVE)
                for n2, fsrc in [("x2", x2f), ("y2", y2f), ("s2", s2f), ("ys", ysf)]:
                    th = persist.tile([P, D], f16, name=f"{n2}h_{bt}", tag=f"{n2}h_{bt}")
                    nc.scalar.activation(th, fsrc, COPY)
                    hi[n2].append(th)
                    tm = persist.tile([P, D], f16, name=f"{n2}m_{bt}", tag=f"{n2}m_{bt}")
                    nc.vector.tensor_tensor(tm, fsrc, th, SUB)
                    mi[n2].append(tm)

                # colsum-only products: fp32 product, split, stream, discard
                # (sequential lifetimes -> shared tags with a few rotating bufs)
                cs_scratch = [
                    ("y3", y2f, yf), ("y2s", y2f, sf), ("ys2", ysf, sf),
                    ("s3", s2f, sf), ("x3", x2f, xf),
                ]
                sc_splits = {}
                for n3, a3, b3 in cs_scratch:
                    pf = scratch.tile([P, D], f32, name=f"{n3}f_{bt}", tag="csf", bufs=3)
                    nc.vector.tensor_tensor(pf, a3, b3, MULT)
                    th = scratch.tile([P, D], f16, name=f"{n3}h_{bt}", tag="csh", bufs=3)
                    nc.scalar.activation(th, pf, COPY)
                    tm = scratch.tile([P, D], f16, name=f"{n3}m_{bt}", tag="csm", bufs=3)
                    nc.gpsimd.tensor_tensor(tm, pf, th, SUB)
                    sc_splits[n3] = (th, tm)

                # stream this bt's 12 sources (hi+mid) into the colsum bank
                for row, srcname in enumerate(COL_SRC):
                    if srcname in sc_splits:
                        th, tm = sc_splits[srcname]
                    else:
                        th, tm = hi[srcname][bt], mi[srcname][bt]
                    cs_mm(row, th)
                    cs_mm(row, tm)

            # ---- dense triples: 3-pass fp16 split matmuls ----
            for m in range(4):
                t0m = persist.tile([P, D], f32, name=f"t0_{m}", tag=f"t0_{m}")
                t1m = persist.tile([P, D], f32, name=f"t1_{m}", tag=f"t1_{m}")
                msl = slice(m * P, (m + 1) * P)
                for ti in range(len(DENSE)):
                    la, ra = DENSE_LHS[ti], DENSE_RHS[ti]
                    ps = ps_dense.tile([P, D], f32, name=f"ps_d_{m}_{ti}", tag="dense")
                    k = 0
                    for bt in range(nbt):
                        for A, X in (
                            (hi[la][bt], hi[ra][bt]),
                            (hi[la][bt], mi[ra][bt]),
                            (mi[la][bt], hi[ra][bt]),
                        ):
                            nc.tensor.matmul(
                                ps, A[:, msl], X,
                                start=(k == 0), stop=(k == 3 * nbt - 1),
                            )
                            k += 1
                    csl = coef_bc[:, ti : ti + 1]
                    if ti == 0:
                        nc.vector.tensor_scalar_mul(t0m, ps, csl)
                        nc.vector.tensor_scalar_mul(t1m, ps, coef_bc[:, 7:8])
                    else:
                        nc.vector.scalar_tensor_tensor(t0m, ps, csl, t0m, MULT, ADD)
                        if ti == 1:
                            nc.vector.scalar_tensor_tensor(
                                t1m, ps, coef_bc[:, 8:9], t1m, MULT, ADD
                            )
                # scatter the [128, D] chunks into the slab-interleaved layout
                for t_src, base in ((t0m, 0), (t1m, RPC)):
                    q = m * P
                    while q < (m + 1) * P:
                        r = q // RPC
                        off = q % RPC
                        n = min(RPC - off, (m + 1) * P - q)
                        nc.sync.dma_start(
                            pview[r, base + off : base + off + n, :],
                            t_src[q - m * P : q - m * P + n, :],
                        )
                        q += n

            # ---- fold colsums into u/g/v vectors ----
            nc.vector.tensor_copy(cp_sb[0:12, :], ps_cs)
            ps_vec = ps_small.tile([16, D], f32, name="ps_vec", tag="small")
            nc.tensor.matmul(ps_vec, ugv_sb, cp_sb, start=True, stop=True)
            vec_sb = persist.tile([16, D], f32, name="vec_sb")
            nc.vector.tensor_copy(vec_sb, ps_vec)
            for r in range(n_cores):
                # u rows pre-sliced per destination rank; g/v replicated.
                # Fill the unread tail of the u rows with ones first (keeps the
                # ReduceScatter input fully finite).
                if RPC < D:
                    nc.sync.dma_start(
                        pview[r, 2 * RPC : 2 * RPC + 3, RPC:D],
                        cp_sb[13:16, RPC:D],
                    )
                nc.sync.dma_start(
                    pview[r, 2 * RPC : 2 * RPC + 3, 0:RPC],
                    vec_sb[0:3, r * RPC : (r + 1) * RPC],
                )
                nc.sync.dma_start(
                    pview[r, 2 * RPC + 3 : 2 * RPC + 10, :], vec_sb[4:11, :]
                )

            # ---- ReduceScatter: rank r receives slab r summed over ranks ----
            nc.gpsimd.collective_compute(
                "ReduceScatter",
                ADD,
                replica_groups=[list(range(n_cores))],
                ins=[part_d.ap().opt()],
                outs=[part_r.ap().opt()],
            )

            # ---- post-RS: combine this rank's RPC rows of dw ----
            # post-RS tiles reuse dense-phase tags (disjoint lifetimes); only
            # when the full-size config makes SBUF tight
            _slots = (
                [f"t0_{q}" for q in range(4)]
                + [f"t1_{q}" for q in range(4)]
                + ["cp_sb", "vec_sb"]
            )
            rt = 0

            def rtile(shape, name):
                nonlocal rt
                tag = _slots[rt] if nch == 1 and rt < len(_slots) else f"pr_{rt}"
                t = persist.tile(shape, f32, name=name, tag=tag)
                rt += 1
                return t

            t0r, t1r = [], []
            for j in range(nch):
                t = rtile([CH, D], f"t0r_{j}")
                nc.sync.dma_start(t, part_r[j * CH : (j + 1) * CH, :])
                t0r.append(t)
                t = rtile([CH, D], f"t1r_{j}")
                nc.sync.dma_start(t, part_r[RPC + j * CH : RPC + (j + 1) * CH, :])
                t1r.append(t)

            nc.sync.dma_start(uin[0:3, :], part_r[2 * RPC : 2 * RPC + 3, 0:RPC])
            ucol = []
            for j in range(nch):
                ps_tr = ps_small.tile([CH, P], f32, name=f"ps_tr_{j}", tag="small")
                nc.tensor.transpose(ps_tr, uin[:, j * CH : (j + 1) * CH], identity)
                uc = persist.tile([CH, 3], f32, name=f"ucol_{j}", tag=f"ucol_{j}")
                nc.vector.tensor_copy(uc, ps_tr[:, 0:3])
                ucol.append(uc)

            vbc = []
            for k in range(3):
                t = rtile([CH, D], f"vbc_{k}")
                nc.sync.dma_start(
                    t,
                    part_r[2 * RPC + 7 + k : 2 * RPC + 8 + k, :].to_broadcast((CH, D)),
                )
                vbc.append(t)

            grow = []
            for k in range(4):
                t = rtile([1, D], f"g_{k}")
                nc.sync.dma_start(t, part_r[2 * RPC + 3 + k : 2 * RPC + 4 + k, :])
                grow.append(t)

            # ---- db (every core computes the full vector; host reads core 0) ----
            dbt = rtile([1, D], "dbt")
            nc.vector.tensor_tensor(dbt, grow[3], b_sb, MULT)
            nc.vector.tensor_tensor(dbt, dbt, grow[2], ADD)
            nc.vector.tensor_tensor(dbt, dbt, b_sb, MULT)
            nc.vector.tensor_tensor(dbt, dbt, grow[1], ADD)
            nc.vector.tensor_tensor(dbt, dbt, b_sb, MULT)
            nc.vector.tensor_tensor(dbt, dbt, grow[0], ADD)
            nc.sync.dma_start(db_d[:, :], dbt)

            # ---- dw Horner per chunk ----
            for j in range(nch):
                g0m = scratch.tile([CH, D], f32, name=f"g0m_{j}", tag="xf", bufs=2)
                g1m = scratch.tile([CH, D], f32, name=f"g1m_{j}", tag="yf", bufs=2)
                nc.vector.scalar_tensor_tensor(
                    g0m, vbc[0], ucol[j][:, 0:1], t0r[j], ADD, ADD
                )
                nc.vector.scalar_tensor_tensor(
                    g1m, vbc[1], ucol[j][:, 1:2], t1r[j], ADD, ADD
                )
                h = scratch.tile([CH, D], f32, name=f"h_{j}", tag="sf", bufs=2)
                nc.vector.tensor_scalar(h, vbc[2], ucol[j][:, 2:3], None, ADD)
                nc.vector.scalar_tensor_tensor(h, wt[j], coef_bc[0:CH, 9:10], h, MULT, ADD)
                nc.vector.tensor_tensor(h, h, wt[j], MULT)
                nc.vector.tensor_tensor(h, h, g1m, ADD)
                nc.vector.tensor_tensor(h, h, wt[j], MULT)
                nc.vector.tensor_tensor(h, h, g0m, ADD)
                nc.sync.dma_start(dw_d[j * CH : (j + 1) * CH, :], h)

    nc.compile()
    return nc


_CACHE = {}


def _get_nc(n_cores, Bs):
    key = (n_cores, Bs)
    if key not in _CACHE:
        _CACHE[key] = build_nc(n_cores, Bs)
    return _CACHE[key]


def _split16(a):
    h = a.astype(np.float16)
    m = (a - h.astype(np.float32)).astype(np.float16)
    return h, m


def run(x, s, y, w, b, params_w, params_b, trace=False):
    from concourse.bass_utils import run_bass_kernel_spmd

    n_cores = N_CORES
    Bs = x.shape[0] // n_cores
    RPC = D // n_cores
    nc = _get_nc(n_cores, Bs)
    coefs, ugv = host_tables(np.asarray(params_w), np.asarray(params_b), n_cores)

    x = np.ascontiguousarray(x, np.float32)
    s = np.ascontiguousarray(s, np.float32)
    y = np.ascontiguousarray(y, np.float32)
    w = np.ascontiguousarray(w, np.float32)
    xh, xm = _split16(x)
    yh, ym = _split16(y)
    sh, sm = _split16(s)
    eye = np.eye(12, dtype=np.float16)
    in_maps = []
    for c in range(n_cores):
        sl = slice(c * Bs, (c + 1) * Bs)
        in_maps.append(
            {
                "xf": x[sl], "yf": y[sl], "sf": s[sl],
                "xh": xh[sl], "xm": xm[sl],
                "yh": yh[sl], "ym": ym[sl],
                "sh": sh[sl], "sm": sm[sl],
                "w": np.ascontiguousarray(w[c * RPC : (c + 1) * RPC]),
                "b": np.ascontiguousarray(b, np.float32).reshape(1, D),
                "coefs": coefs,
                "ugv": ugv,
                "eye12": eye,
            }
        )
    res = run_bass_kernel_spmd(
        nc, in_maps, core_ids=list(range(n_cores)), trace=trace
    )
    dw = np.concatenate([res.results[c]["dw"] for c in range(n_cores)], axis=0)
    db = res.results[0]["db"].reshape(D)
    return dw, db, res


def kernel(x, s, y, w, b, params_w, params_b):
    dw, db, _ = run(x, s, y, w, b, params_w, params_b)
    return dw, db
